# revision 1
# baseline (speedup 1.0000x reference)
import numpy as np
import jax
import jax.numpy as jnp

# nn_MAB: B=256, Npt=25, Sd=10, T=40, C=64, inter=16, D=2560, 8 heads.
# Pure data parallel: batch 256 -> 32 per core across 8 NeuronCores.

NUM_SUBSET = 3
BN_EPS = 1e-5
T_CONST = 40
NUM_HEADS = 8
NCORES = 8


def _unit_gcn(x, PA, Wa, ba, Wb, bb, Wd, bd, gamma, beta):
    B, C, T, V = x.shape
    inter = Wa.shape[1]
    x_flat = x.reshape(B, C * T, V)
    y = None
    for i in range(NUM_SUBSET):
        a = jnp.einsum('bctv,ic->bitv', x, Wa[i]) + ba[i][None, :, None, None]
        A1 = a.transpose(0, 3, 1, 2).reshape(B, V, inter * T)
        b = jnp.einsum('bctv,ic->bitv', x, Wb[i]) + bb[i][None, :, None, None]
        A2 = b.reshape(B, inter * T, V)
        S = jax.nn.softmax((A1 @ A2) / (inter * T), axis=-2) + PA[i]
        z = (x_flat @ S).reshape(B, C, T, V)
        z = jnp.einsum('bctv,oc->botv', z, Wd[i]) + bd[i][None, :, None, None]
        y = z if y is None else y + z
    y = y * (gamma / jnp.sqrt(1.0 + BN_EPS))[None, :, None, None] + beta[None, :, None, None]
    y = y + x
    return jax.nn.relu(y)


def _mab_forward(Q, K, fck, fcv, fco):
    B, Npt, DK = K.shape
    T = T_CONST
    C = DK // T
    Kr = K.transpose(0, 2, 1).reshape(B, C, T, Npt)
    Kg = _unit_gcn(Kr, *fck)
    Vg = _unit_gcn(Kr, *fcv)
    Kf = Kg.transpose(0, 3, 1, 2).reshape(B, Npt, -1)
    Vf = Vg.transpose(0, 3, 1, 2).reshape(B, Npt, -1)
    S, DV = Q.shape[1], Q.shape[2]
    ds = DV // NUM_HEADS
    Qh = Q.reshape(B, S, NUM_HEADS, ds)
    Kh = Kf.reshape(B, Npt, NUM_HEADS, ds)
    Vh = Vf.reshape(B, Npt, NUM_HEADS, ds)
    scores = jnp.einsum('bqhd,bkhd->bhqk', Qh, Kh) / jnp.sqrt(jnp.float32(DV))
    attn = jax.nn.softmax(scores, axis=-1)
    Oh = Qh + jnp.einsum('bhqk,bkhd->bqhd', attn, Vh)
    O = Oh.reshape(B, S, DV)
    Og = O.transpose(0, 2, 1).reshape(B, C, T, S)
    Og = _unit_gcn(Og, *fco)
    Og = Og.transpose(0, 3, 1, 2).reshape(B, S, -1)
    return O + jax.nn.relu(Og)


_FCK = ('PA', 'Wa', 'ba', 'Wb', 'bb', 'Wd', 'bd', 'gamma', 'beta')


def _shard_fn(Q, K, params):
    fck = tuple(params['fck_' + n] for n in _FCK)
    fcv = tuple(params['fcv_' + n] for n in _FCK)
    fco = tuple(params['fco_' + n] for n in _FCK)
    return _mab_forward(Q, K, fck, fcv, fco)


_pmapped = None


def _get_pmapped():
    global _pmapped
    if _pmapped is None:
        _pmapped = jax.pmap(_shard_fn, in_axes=(0, 0, None), devices=jax.devices()[:NCORES])
    return _pmapped


def kernel(**inputs):
    Q = np.asarray(inputs['Q'], np.float32)
    K = np.asarray(inputs['K'], np.float32)
    B = Q.shape[0]
    params = {k: jnp.asarray(v) for k, v in inputs.items()
              if k.startswith(('fck_', 'fcv_', 'fco_'))}
    per = B // NCORES
    Qs = Q.reshape(NCORES, per, Q.shape[1], Q.shape[2])
    Ks = K.reshape(NCORES, per, K.shape[1], K.shape[2])
    out = _get_pmapped()(Qs, Ks, params)
    out = np.asarray(out)
    return out.reshape(B, out.shape[2], out.shape[3]).astype(np.float32)
